# revision 1
# baseline (speedup 1.0000x reference)
"""Trainium2 Bass kernel for nn_Conv_39273180955618.

The reference op reduces to a depthwise correlation: every image (batch x
channel plane) of X is correlated with the same 3x3 kernel
Keff = K.sum((0,1)), plus a scalar bias b * prod(K.shape).

Strategy (8 NeuronCores, data-parallel over batch):
  - core k gets batches [2k, 2k+2) = 128 images of 224x224.
  - Per core, images are processed in blocks of IB images x 112-row chunks.
    Rows live on SBUF partitions, W stays contiguous on the free axis.
  - The H-convolution is a TensorE matmul contraction over rows with small
    banded matrices B[chunk, dw] (shape [113, 112]): for each of the 3 W
    shifts dw, Z[:, wout] += B^T @ X[rows, win], accumulated in PSUM.
    H zero-padding is folded into the band matrices, W zero-padding into
    the matmul column ranges.
  - fp32 data is fed to the PE as float32r (full-rate fp32 matmul mode).
  - PSUM -> SBUF eviction (+ bias) alternates between ScalarE and VectorE,
    and DMA in/out transfers are ~1.6 MB each for near-peak HBM bandwidth.
"""

import numpy as np

import bass_rust
import concourse.bass as bass
import concourse.mybir as mybir
import concourse.tile as tile
from concourse.bass_utils import run_bass_kernel_spmd

F32 = mybir.dt.float32
F32R = mybir.dt.float32r

N_CORES = 8
H = W = 224
M = 112        # output rows per chunk
KR = 113       # input rows per chunk (M + 1 halo row at the image edge)
IMGS = 128     # images per core (2 batches x 64 channels)
IB = 32        # images per block (DMA granularity)
NBLK = IMGS // IB
WP = W + 2     # padded image-row width in SBUF (zero column at each edge)
NWIN = 2 * WP - 2  # flat matmul window: 2 images per PSUM group, minus 2
# (r0, i0) per chunk: output-row base and input-row base.
CHUNKS = ((0, 0), (112, 111))

_MAX_WAITS = 1


def _split_multi_waits(nc):
    """Split instructions carrying >1 sync-wait into single-wait NOP
    preludes (the walrus build here rejects multi-wait instructions)."""
    counter = 0
    for fn in nc.m.functions:
        for bb in fn.blocks:
            insts = bb.instructions
            i = 0
            while i < len(insts):
                inst = insts[i]
                si = inst.sync_info
                if si is not None and si.on_wait and len(si.on_wait) > _MAX_WAITS:
                    waits = list(si.on_wait)
                    keep = waits[-_MAX_WAITS:]
                    spill = waits[:-_MAX_WAITS]
                    nops = []
                    for w in spill:
                        nop = mybir.InstNoOp(
                            name=f"waitsplit_{counter}", ins=[], outs=[]
                        )
                        counter += 1
                        nop.engine = inst.engine
                        nop.sync_info = bass_rust.SyncInfo(on_wait=[w], on_update=[])
                        nops.append(nop)
                    inst.sync_info = bass_rust.SyncInfo(
                        on_wait=keep,
                        on_update=list(si.on_update) if si.on_update else [],
                    )
                    insts[i:i] = nops
                    i += len(nops)
                i += 1
    return counter


def build_nc(bias_total: float):
    nc = bass.Bass("TRN2", target_bir_lowering=False, debug=False)
    x_d = nc.dram_tensor("X", [IMGS, H, WP], F32R, kind="ExternalInput").ap()
    bands_d = nc.dram_tensor("BANDS", [2, 3, KR, M], F32R, kind="ExternalInput").ap()
    y_d = nc.dram_tensor("Y", [IMGS, H, W], F32, kind="ExternalOutput").ap()

    with tile.TileContext(nc) as tc:
        with (
            tc.tile_pool(name="const", bufs=1) as cpool,
            tc.tile_pool(name="io", bufs=3) as io_pool,
            tc.tile_pool(name="acc", bufs=8, space="PSUM") as psum_pool,
        ):
            bands = cpool.tile([KR, 2, 3, M], F32R)
            nc.sync.dma_start(bands, bands_d.rearrange("c s k m -> k c s m"))
            ev = 0
            for blk in range(NBLK):
                for c, (r0, i0) in enumerate(CHUNKS):
                    xt = io_pool.tile([KR, IB, WP], F32R, tag="xt")
                    # X arrives host-padded to 226 columns (zero at each
                    # edge), so the DMA delivers the W padding directly.
                    nc.sync.dma_start(
                        xt,
                        x_d[blk * IB:(blk + 1) * IB, i0:i0 + KR, :].rearrange(
                            "i r w -> r i w"
                        ),
                    )
                    xtf = xt.rearrange("k i w -> k (i w)")
                    ot = io_pool.tile([M, IB, W], F32, tag="ot")
                    for p in range(IB // 2):
                        base = 2 * p * WP
                        # One flat 450-wide window per W-shift: fp32r matmuls
                        # need a single even-count free dim and an 8B-aligned
                        # PSUM dst at offset 0, so the dst is always [:, 0:450]
                        # and the W-shift slides the source window. PSUM
                        # columns 224/225 catch the inter-image junk and are
                        # not evicted.
                        ps = psum_pool.tile([M, 2 * WP], F32)
                        for k, dw in enumerate((0, 1, 2)):
                            nc.tensor.matmul(
                                ps[:, 0:NWIN],
                                bands[:, c, dw, :],
                                xtf[:, base + dw:base + dw + NWIN],
                                start=(k == 0),
                                stop=(k == 2),
                            )
                        psv = ps.rearrange("m (i w) -> m i w", w=WP)[:, :, 0:W]
                        dst = ot[:, 2 * p:2 * p + 2, :]
                        if ev % 2 == 0:
                            if bias_total != 0.0:
                                nc.scalar.activation(
                                    dst,
                                    psv,
                                    mybir.ActivationFunctionType.Copy,
                                    bias=float(bias_total),
                                )
                            else:
                                nc.scalar.copy(dst, psv)
                        else:
                            if bias_total != 0.0:
                                nc.vector.tensor_scalar_add(
                                    dst, psv, float(bias_total)
                                )
                            else:
                                nc.vector.tensor_copy(dst, psv)
                        ev += 1
                        # Stores go on the ACT HWDGE ring so the next
                        # block's load (SP ring) never queues behind this
                        # store's eviction wait; two half-stores per block
                        # let the store pipeline start after 4 evictions.
                        if p % 4 == 3:
                            h0 = (p - 3) * 2
                            nc.scalar.dma_start(
                                y_d[
                                    blk * IB + h0:blk * IB + h0 + 8,
                                    r0:r0 + M,
                                    :,
                                ].rearrange("i r w -> r i w"),
                                ot[:, h0:h0 + 8, :],
                            )
    _split_multi_waits(nc)
    return nc


def build_bands(Keff: np.ndarray) -> np.ndarray:
    """Banded H-contraction matrices, [chunk, dw, KR, M] fp32.

    B[c, dw, i, m] = Keff[dh, dw] where input-row index i corresponds to
    absolute row i0 + i and output row r0 + m needs absolute row
    r0 + m + dh - 1; rows outside [0, H) are dropped (zero padding).
    """
    bands = np.zeros((2, 3, KR, M), dtype=np.float32)
    for c, (r0, i0) in enumerate(CHUNKS):
        for dw in range(3):
            for m in range(M):
                for dh in range(3):
                    arow = r0 + m + dh - 1
                    if 0 <= arow < H:
                        bands[c, dw, arow - i0, m] = Keff[dh, dw]
    return bands


_cache = {}


def kernel(X, K, b, padding, stride) -> np.ndarray:
    X = np.ascontiguousarray(np.asarray(X, dtype=np.float32))
    K = np.asarray(K, dtype=np.float32)
    b = np.asarray(b, dtype=np.float32)
    assert int(padding) == 1 and int(stride) == 1, (padding, stride)
    bx, cx, hx, wx = X.shape
    assert (bx, cx, hx, wx) == (16, 64, H, W), X.shape

    bk, ck, hk, wk = K.shape
    Keff = K.sum(axis=(0, 1), dtype=np.float32)
    bias_total = float(b.reshape(())) * (bk * ck * hk * wk)

    key = (round(bias_total, 12) != 0.0)
    if key not in _cache:
        _cache[key] = build_nc(bias_total)
    nc = _cache[key]

    bands = build_bands(Keff)
    Xf = X.reshape(bx * cx, hx, wx)
    Xp = np.zeros((bx * cx, hx, WP), dtype=np.float32)
    Xp[:, :, 1:1 + W] = Xf
    in_maps = [
        {
            "X": Xp[k * IMGS:(k + 1) * IMGS],
            "BANDS": bands,
        }
        for k in range(N_CORES)
    ]
    res = run_bass_kernel_spmd(nc, in_maps, core_ids=list(range(N_CORES)))
    out = np.concatenate([r["Y"] for r in res.results], axis=0)
    return out.reshape(bx, cx, hx, wx)



# revision 2
# speedup vs baseline: 1.7712x; 1.7712x over previous
"""Trainium2 Bass kernel for nn_Conv_39273180955618.

The reference op reduces to a depthwise correlation: every image (batch x
channel plane) of X is correlated with the same 3x3 kernel
Keff = K.sum((0,1)), plus a scalar bias b * prod(K.shape).

Strategy (8 NeuronCores, data-parallel over batch):
  - core k gets batches [2k, 2k+2) = 128 images of 224x224.
  - All device traffic is fp16 (X quantized on host, Y dequantized on
    host); rel tolerance is 2e-2 and fp16 end-to-end lands ~5e-4.
    This halves HBM/DMA bytes, the bottleneck for this kernel.
  - DRAM layout is [row, image, w] so each DMA descriptor moves a
    >=512B contiguous run (below 512B the DMA pays a 2x penalty).
  - Per core, images are processed in blocks of IB images x 112-row chunks.
    Rows live on SBUF partitions, W stays contiguous on the free axis.
  - The H-convolution is a TensorE matmul contraction over rows with small
    banded matrices B[chunk, dw] (shape [113, 112]): for each of the 3 W
    shifts dw, Z[:, wout] += B^T @ X[rows, win], accumulated in PSUM.
    H zero-padding is folded into the band matrices, W zero-padding into
    the host-padded 226-wide rows.
  - PSUM -> SBUF eviction (+ bias, fp32->fp16) alternates between ScalarE
    and VectorE; loads ride the SP ring, stores the ACT ring so neither
    queues behind the other.
"""

import numpy as np

import bass_rust
import concourse.bass as bass
import concourse.mybir as mybir
import concourse.tile as tile
from concourse.bass_utils import run_bass_kernel_spmd

F32 = mybir.dt.float32
F16 = mybir.dt.float16

N_CORES = 8
H = W = 224
M = 112        # output rows per chunk
KR = 113       # input rows per chunk (M + 1 halo row at the image edge)
IMGS = 128     # images per core (2 batches x 64 channels)
IB = 16        # images per block (DMA granularity)
NBLK = IMGS // IB
WP = W + 2     # padded image-row width (zero column at each edge)
NWIN = 2 * WP - 2  # flat matmul window: 2 images per PSUM group, minus 2
# (r0, i0) per chunk: output-row base and input-row base.
CHUNKS = ((0, 0), (112, 111))

_MAX_WAITS = 1


def _split_multi_waits(nc):
    """Split instructions carrying >1 sync-wait into single-wait NOP
    preludes (the walrus build here rejects multi-wait instructions)."""
    counter = 0
    for fn in nc.m.functions:
        for bb in fn.blocks:
            insts = bb.instructions
            i = 0
            while i < len(insts):
                inst = insts[i]
                si = inst.sync_info
                if si is not None and si.on_wait and len(si.on_wait) > _MAX_WAITS:
                    waits = list(si.on_wait)
                    keep = waits[-_MAX_WAITS:]
                    spill = waits[:-_MAX_WAITS]
                    nops = []
                    for w in spill:
                        nop = mybir.InstNoOp(
                            name=f"waitsplit_{counter}", ins=[], outs=[]
                        )
                        counter += 1
                        nop.engine = inst.engine
                        nop.sync_info = bass_rust.SyncInfo(on_wait=[w], on_update=[])
                        nops.append(nop)
                    inst.sync_info = bass_rust.SyncInfo(
                        on_wait=keep,
                        on_update=list(si.on_update) if si.on_update else [],
                    )
                    insts[i:i] = nops
                    i += len(nops)
                i += 1
    return counter


def build_nc(bias_total: float):
    nc = bass.Bass("TRN2", target_bir_lowering=False, debug=False)
    # X arrives host-transposed to [row, image, w] fp16 with a zero column
    # at each W edge so DMA runs stay contiguous and >=512B.
    x_d = nc.dram_tensor("X", [H, IMGS, WP], F16, kind="ExternalInput").ap()
    bands_d = nc.dram_tensor("BANDS", [KR, 2, 3, M], F16, kind="ExternalInput").ap()
    y_d = nc.dram_tensor("Y", [H, IMGS, W], F16, kind="ExternalOutput").ap()

    with tile.TileContext(nc) as tc:
        with (
            tc.tile_pool(name="const", bufs=1) as cpool,
            tc.tile_pool(name="io", bufs=3) as io_pool,
            tc.tile_pool(name="acc", bufs=8, space="PSUM") as psum_pool,
        ):
            bands = cpool.tile([KR, 2, 3, M], F16)
            nc.sync.dma_start(bands, bands_d)
            ev = 0
            for blk in range(NBLK):
                for c, (r0, i0) in enumerate(CHUNKS):
                    xt = io_pool.tile([KR, IB, WP], F16, tag="xt")
                    nc.sync.dma_start(
                        xt, x_d[i0:i0 + KR, blk * IB:(blk + 1) * IB, :]
                    )
                    xtf = xt.rearrange("k i w -> k (i w)")
                    ot = io_pool.tile([M, IB, W], F16, tag="ot")
                    for p in range(IB // 2):
                        base = 2 * p * WP
                        # One flat 450-wide window per W-shift; PSUM columns
                        # 224/225 catch the inter-image junk and are not
                        # evicted.
                        ps = psum_pool.tile([M, 2 * WP], F32)
                        for k, dw in enumerate((0, 1, 2)):
                            nc.tensor.matmul(
                                ps[:, 0:NWIN],
                                bands[:, c, dw, :],
                                xtf[:, base + dw:base + dw + NWIN],
                                start=(k == 0),
                                stop=(k == 2),
                            )
                        psv = ps.rearrange("m (i w) -> m i w", w=WP)[:, :, 0:W]
                        dst = ot[:, 2 * p:2 * p + 2, :]
                        if ev % 2 == 0:
                            if bias_total != 0.0:
                                nc.scalar.activation(
                                    dst,
                                    psv,
                                    mybir.ActivationFunctionType.Copy,
                                    bias=float(bias_total),
                                )
                            else:
                                nc.scalar.copy(dst, psv)
                        else:
                            if bias_total != 0.0:
                                nc.vector.tensor_scalar_add(
                                    dst, psv, float(bias_total)
                                )
                            else:
                                nc.vector.tensor_copy(dst, psv)
                        ev += 1
                        # Stores go on the ACT ring so the next block's load
                        # (SP ring) never queues behind this store's
                        # eviction wait; two half-stores per chunk.
                        if p % 4 == 3:
                            h0 = (p - 3) * 2
                            nc.scalar.dma_start(
                                y_d[
                                    r0:r0 + M,
                                    blk * IB + h0:blk * IB + h0 + 8,
                                    :,
                                ],
                                ot[:, h0:h0 + 8, :],
                            )
    _split_multi_waits(nc)
    return nc


def build_bands(Keff: np.ndarray) -> np.ndarray:
    """Banded H-contraction matrices, [KR, chunk, dw, M] fp16.

    B[i, c, dw, m] = Keff[dh, dw] where input-row index i corresponds to
    absolute row i0 + i and output row r0 + m needs absolute row
    r0 + m + dh - 1; rows outside [0, H) are dropped (zero padding).
    """
    bands = np.zeros((KR, 2, 3, M), dtype=np.float32)
    for c, (r0, i0) in enumerate(CHUNKS):
        for dw in range(3):
            for m in range(M):
                for dh in range(3):
                    arow = r0 + m + dh - 1
                    if 0 <= arow < H:
                        bands[arow - i0, c, dw, m] = Keff[dh, dw]
    return bands.astype(np.float16)


_cache = {}


def kernel(X, K, b, padding, stride) -> np.ndarray:
    X = np.asarray(X, dtype=np.float32)
    K = np.asarray(K, dtype=np.float32)
    b = np.asarray(b, dtype=np.float32)
    assert int(padding) == 1 and int(stride) == 1, (padding, stride)
    bx, cx, hx, wx = X.shape
    assert (bx, cx, hx, wx) == (16, 64, H, W), X.shape

    bk, ck, hk, wk = K.shape
    Keff = K.sum(axis=(0, 1), dtype=np.float32)
    bias_total = float(b.reshape(())) * (bk * ck * hk * wk)

    key = (round(bias_total, 12) != 0.0)
    if key not in _cache:
        _cache[key] = build_nc(bias_total)
    nc = _cache[key]

    bands = build_bands(Keff)
    # Host marshalling: fp16 quantize, pad W to 226 with zeros, and lay
    # out each core's shard as [row, image, w].
    Xp = np.zeros((bx * cx, H, WP), dtype=np.float16)
    Xp[:, :, 1:1 + W] = X.reshape(bx * cx, hx, wx)
    in_maps = [
        {
            "X": np.ascontiguousarray(
                Xp[k * IMGS:(k + 1) * IMGS].transpose(1, 0, 2)
            ),
            "BANDS": bands,
        }
        for k in range(N_CORES)
    ]
    res = run_bass_kernel_spmd(nc, in_maps, core_ids=list(range(N_CORES)))
    out = np.concatenate(
        [r["Y"].transpose(1, 0, 2) for r in res.results], axis=0
    )
    return out.astype(np.float32).reshape(bx, cx, hx, wx)
